# revision 26
# baseline (speedup 1.0000x reference)
"""DeepSeek-style MLA decode attention (batch=8, 128 heads, cache 512) on 8 NeuronCores.

Sharding: tensor-parallel over heads (16 heads/core).
 - down-projection [Wq_down | Wkv_down] sharded by OUTPUT column (256 cols/core);
   each core computes its cdown column slice (transposed) and a tiny AllGather
   (256x8 f32) replicates the full cdownT to every core.
 - Wq_up / Wv_up / k_cache / v_cache / Wo sharded by head.
 - o_proj partials ReduceScattered over batch (core b returns batch b's row).

All big streams are cast on the host: weights bf16, wq_up + kv caches fp8(e4m3).
Every matmul keeps the large tensor as the STATIONARY operand (it has to pass
through the PE array exactly once either way) and streams the 8-wide batch as
the moving operand, so every intermediate comes out feature-major ("pre-
transposed") and chains straight into the next matmul: no transposes and no
per-row extraction anywhere.

Note: the reference's "new token" softmax is over a length-1 axis (== 1.0), so
k_new/Wk_up are dead and the new-token contribution is simply + v_new.
"""

import numpy as np
import ml_dtypes

import concourse.bass as bass
import concourse.mybir as mybir
import concourse.tile as tile
from concourse import bacc
from concourse import bass_utils

NC_ = 8                      # cores
B = 8                        # batch
H = 128                      # total heads
HP = H // NC_                # 16 heads per core
D = 128                      # head dim
L = 512                      # cache len
HID = 7168
QL = 1536
KVL = 512
COLS = QL + KVL              # 2048 down-proj output cols
COLP = COLS // NC_           # 256 cols per core
NH = HP * D                  # 2048 per-core head cols
SCALE = 1.0 / float(np.sqrt(D))

F32 = mybir.dt.float32
BF = mybir.dt.bfloat16

# fp8 (e4m3) for the attention streams; weights are scaled on the host to sit
# in e4m3's normal range and the inverse scale is folded into the softmax /
# copy scales below.
USE_FP8_KV = True            # kt, v, qT, probsT
USE_FP8_WQ = True            # wq_up, cqT
KVDT = mybir.dt.float8e4 if USE_FP8_KV else BF
WQDT = mybir.dt.float8e4 if USE_FP8_WQ else BF
WQ_SCALE = 16.0 if USE_FP8_WQ else 1.0   # host multiplies Wq_up by this
QT_SCALE = (1.0 / 32.0) if USE_FP8_KV else 1.0  # applied when casting qT
# scoresT = (WQ_SCALE * QT_SCALE) * true_score
EXP_SCALE = SCALE / (WQ_SCALE * QT_SCALE)

NP_BF16 = ml_dtypes.bfloat16
NP_FP8 = ml_dtypes.float8_e4m3


def build_nc():
    nc = bacc.Bacc(
        "TRN2",
        target_bir_lowering=False,
        debug=False,
        enable_asserts=True,
        num_devices=NC_,
    )
    xt = nc.dram_tensor("xt", [128, 56 * B], BF, kind="ExternalInput").ap()
    w_down = nc.dram_tensor("w_down", [128, 56 * COLP], BF, kind="ExternalInput").ap()
    wq_up = nc.dram_tensor("wq_up", [12, 128, NH], WQDT, kind="ExternalInput").ap()
    wv_up = nc.dram_tensor("wv_up", [4, 128, NH], BF, kind="ExternalInput").ap()
    kt = nc.dram_tensor("kt", [32, 128, 2048], KVDT, kind="ExternalInput").ap()
    v = nc.dram_tensor("v", [32, 128, 2048], KVDT, kind="ExternalInput").ap()
    wo = nc.dram_tensor("wo", [NH, HID], BF, kind="ExternalInput").ap()
    o = nc.dram_tensor("o", [1, HID], BF, kind="ExternalOutput").ap()

    rg = [list(range(NC_))]

    with tile.TileContext(nc) as tc:
        with (
            tc.tile_pool(name="const", bufs=1) as constp,
            tc.tile_pool(name="sbuf", bufs=1) as sb,
            tc.tile_pool(name="stage", bufs=2) as stg,
            tc.tile_pool(name="wdp", bufs=4) as wdp,
            tc.tile_pool(name="wqp", bufs=12) as wqp,
            tc.tile_pool(name="wvp", bufs=4) as wvp,
            tc.tile_pool(name="ktp", bufs=22) as ktp,
            tc.tile_pool(name="vp", bufs=22) as vp,
            tc.tile_pool(name="wop", bufs=14) as wop,
            tc.tile_pool(name="psA", bufs=2, space="PSUM") as psA,
            tc.tile_pool(name="psB", bufs=6, space="PSUM") as psB,
            tc.tile_pool(name="dram", bufs=1, space="DRAM") as dram,
        ):
            # ---------------- load x and the down-proj column shard ----------------
            xt_sb = constp.tile([128, 56 * B], BF, tag="xt")
            nc.sync.dma_start(out=xt_sb[:], in_=xt[:])

            # ---------------- cdownT column slice: [COLP, B] ----------------
            # cdT[n, b] = sum_hid Wcat[hid, c*COLP+n] * x[b, hid]
            # w_down streams through a small pool: tile s holds i-chunks 4s..4s+3.
            ps_cds = [
                psB.tile([128, 512], F32, tag="bank", name=f"ps_cd{half}")
                for half in range(2)
            ]
            cd_sb = sb.tile([128, 16], F32, tag="cd")
            for s in range(14):
                wd_t = wdp.tile([128, 4 * COLP], BF, tag="wd")
                nc.sync.dma_start(
                    out=wd_t[:], in_=w_down[:, s * 4 * COLP:(s + 1) * 4 * COLP]
                )
                for q in range(4):
                    i = s * 4 + q
                    for half in range(2):
                        nc.tensor.matmul(
                            ps_cds[half][0:128, 0:B],
                            wd_t[:, q * COLP + half * 128:q * COLP + (half + 1) * 128],
                            xt_sb[:, i * B:(i + 1) * B],
                            start=(i == 0), stop=(i == 55),
                        )
            for half in range(2):
                nc.vector.tensor_copy(
                    cd_sb[:, half * B:(half + 1) * B], ps_cds[half][0:128, 0:B]
                )

            cd_bounce = dram.tile([COLP, B], F32, tag="cdb")
            nc.sync.dma_start(
                out=cd_bounce[:].rearrange("(h p) b -> p h b", p=128),
                in_=cd_sb[:].rearrange("p (h b) -> p h b", h=2),
            )
            cd_gathered = dram.tile([COLS, B], F32, tag="cdg")
            nc.gpsimd.collective_compute(
                "AllGather",
                mybir.AluOpType.bypass,
                replica_groups=rg,
                ins=[cd_bounce.opt()],
                outs=[cd_gathered.opt()],
            )
            # cdT_sb[p, i*8+b] = cdown col (i*128+p), batch b
            cdT_sb = sb.tile([128, 128], F32, tag="cdT")
            nc.sync.dma_start(
                out=cdT_sb[:].rearrange("p (i b) -> p i b", i=16),
                in_=cd_gathered[:].rearrange("(i p) b -> p i b", p=128),
            )
            # chunks 0..11 = q rank (1536), 12..15 = kv rank (512)
            cqT = sb.tile([128, 96], WQDT, tag="cqT")
            nc.vector.tensor_copy(cqT[:], cdT_sb[:, 0:96])
            ckvT = sb.tile([128, 32], BF, tag="ckvT")
            nc.vector.tensor_copy(ckvT[:], cdT_sb[:, 96:128])

            # ---------------- qT = (cq @ Wq_up_c)^T per head: [128 d, 16h*8b] ----------------
            ps_qT = psB.tile([128, 512], F32, tag="bank", name="ps_qT")
            wq_tiles = []
            for j in range(12):
                wq_t = wqp.tile([128, NH], WQDT, tag="wq")
                nc.sync.dma_start(out=wq_t[:], in_=wq_up[j])
                wq_tiles.append(wq_t)
            for h in range(HP):
                for j in range(12):
                    nc.tensor.matmul(
                        ps_qT[0:128, h * B:(h + 1) * B],
                        wq_tiles[j][:, h * D:(h + 1) * D],
                        cqT[:, j * B:(j + 1) * B],
                        start=(j == 0), stop=(j == 11),
                    )
            qT = sb.tile([128, 128], KVDT, tag="qT")
            nc.scalar.activation(
                qT[:], ps_qT[0:128, 0:128],
                mybir.ActivationFunctionType.Copy, scale=QT_SCALE,
            )

            # ---------------- v_newT per head: [128 d, 16h*8b] ----------------
            ps_vn = psA.tile([128, 512], F32, tag="bank", name="ps_vn")
            wv_tiles = []
            for j in range(4):
                wv_t = wvp.tile([128, NH], BF, tag="wv")
                nc.sync.dma_start(out=wv_t[:], in_=wv_up[j])
                wv_tiles.append(wv_t)
            for h in range(HP):
                for j in range(4):
                    nc.tensor.matmul(
                        ps_vn[0:128, h * B:(h + 1) * B],
                        wv_tiles[j][:, h * D:(h + 1) * D],
                        ckvT[:, j * B:(j + 1) * B],
                        start=(j == 0), stop=(j == 3),
                    )

            # ---------------- phase A: scoresT [128 k, 4j * 128 hb] ----------------
            # kt tile g holds hb=4g..4g+3 as [128 d, (u, l)]; stationary slice
            # [128 d, 128 k] per (hb, j), moving = qT column hb.
            ps_sc = psB.tile([128, 512], F32, tag="bank", name="ps_sc")
            for g in range(32):
                kt_t = ktp.tile([128, 2048], KVDT, tag="kt")
                nc.sync.dma_start(out=kt_t[:], in_=kt[g])
                for u in range(4):
                    hb = 4 * g + u
                    for j in range(4):
                        nc.tensor.matmul(
                            ps_sc[0:128, j * 128 + hb:j * 128 + hb + 1],
                            kt_t[:, u * 512 + j * 128:u * 512 + (j + 1) * 128],
                            qT[:, hb:hb + 1],
                            start=True, stop=True,
                        )

            # softmax pieces: probsT = exp(EXP_SCALE * scoresT) (unnormalized),
            # denom via ones-matmul, reciprocal broadcast to [128, hb].
            # bias shifts exp into fp8 range (max |score*scale| ~ 6); softmax
            # is shift-invariant so the denominator cancels it exactly.
            probsT = sb.tile([128, 512], KVDT, tag="probsT")
            exp_bias = constp.tile([128, 1], F32, tag="exp_bias")
            nc.vector.memset(exp_bias[:], -2.5)
            nc.scalar.activation(
                probsT[:], ps_sc[0:128, 0:512],
                mybir.ActivationFunctionType.Exp, scale=EXP_SCALE, bias=exp_bias[:],
            )
            ones_kv = constp.tile([128, 1], KVDT, tag="ones_kv")
            nc.vector.memset(ones_kv[:], 1.0)
            ps_dn = psB.tile([1, 128], F32, tag="bank", name="ps_dn")
            for j in range(4):
                nc.tensor.matmul(
                    ps_dn[0:1, 0:128],
                    ones_kv[:],
                    probsT[:, j * 128:(j + 1) * 128],
                    start=(j == 0), stop=(j == 3),
                )
            recip = sb.tile([1, 128], F32, tag="recip")
            nc.vector.reciprocal(recip[:], ps_dn[0:1, 0:128])
            ones_f32 = constp.tile([1, 128], F32, tag="ones_f32")
            nc.vector.memset(ones_f32[:], 1.0)
            ps_rb = psB.tile([128, 128], F32, tag="bank", name="ps_rb")
            nc.tensor.matmul(
                ps_rb[0:128, 0:128], ones_f32[:], recip[:], start=True, stop=True,
            )
            rb_sb = sb.tile([128, 128], F32, tag="rb_sb")
            nc.vector.tensor_copy(rb_sb[:], ps_rb[0:128, 0:128])

            # ---------------- phase B: cacheT [128 d, 128 hb] ----------------
            # v tile g holds hb=4g..4g+3 as [128 kk, (u, j, d)]; stationary
            # slice [128 kk, 128 d] per (hb, j), moving = probsT column.
            # ---------------- phase B + C round 0, interleaved per head ----------------
            # After head h's two v tiles finish, combine its attnT columns and
            # immediately emit its round-0 o_proj matmuls: the tensor queue is
            # in-order, so overlap requires interleaved EMISSION, not just
            # ready data. Remaining C rounds follow after B.
            ps_ca = psA.tile([128, 128], F32, tag="bank", name="ps_ca")
            tmp_at = sb.tile([128, 128], F32, tag="tmp_at")
            attnT = sb.tile([128, 128], BF, tag="attnT")
            o_bounce0 = dram.tile([B, 12 * 512], BF, tag="ob0")
            o_bounce1 = dram.tile([B, HID - 12 * 512], BF, tag="ob1")
            ps_os0 = [
                psB.tile([8, 512], F32, tag="bank", name=f"ps_o0_{i}")
                for i in range(4)
            ]
            for g in range(32):
                v_t = vp.tile([128, 2048], KVDT, tag="v")
                nc.sync.dma_start(out=v_t[:], in_=v[g])
                for u in range(4):
                    hb = 4 * g + u
                    for j in range(4):
                        nc.tensor.matmul(
                            ps_ca[0:128, hb:hb + 1],
                            v_t[:, (u * 4 + j) * 128:(u * 4 + j + 1) * 128],
                            probsT[:, j * 128 + hb:j * 128 + hb + 1],
                            start=(j == 0), stop=(j == 3),
                        )
                if g % 2 == 1:
                    # head h = g//2 complete: attnT_h = cacheT_h/denom + v_newT_h
                    hc = g // 2
                    cs = slice(hc * B, (hc + 1) * B)
                    nc.vector.tensor_mul(
                        tmp_at[:, cs], ps_ca[0:128, cs], rb_sb[:, cs]
                    )
                    nc.vector.tensor_add(
                        attnT[:, cs], tmp_at[:, cs], ps_vn[0:128, cs]
                    )
                    # C round 0 (cols 0:2048) for this head
                    wo_t = wop.tile([128, 2048], BF, tag="wo")
                    nc.sync.dma_start(
                        out=wo_t[:], in_=wo[hc * D:(hc + 1) * D, 0:4 * 512]
                    )
                    for i in range(4):
                        nc.tensor.matmul(
                            ps_os0[i][:8, :],
                            attnT[:, cs],
                            wo_t[:, i * 512:(i + 1) * 512],
                            start=(hc == 0), stop=(hc == HP - 1),
                        )
            for i in range(4):
                ostage = stg.tile([8, 512], BF, tag="ostage")
                nc.vector.tensor_copy(ostage[:], ps_os0[i][:8, :])
                nc.sync.dma_start(
                    out=o_bounce0[:, i * 512:(i + 1) * 512], in_=ostage[:]
                )

            # ---------------- phase C rounds 1-3 ----------------
            for n0, n1 in ((4, 8), (8, 12), (12, 14)):
                nn = n1 - n0
                ps_os = [
                    psB.tile([8, 512], F32, tag="bank", name=f"ps_o{n0}_{i}")
                    for i in range(nn)
                ]
                for h in range(HP):
                    wo_t = wop.tile([128, 2048], BF, tag="wo")
                    nc.sync.dma_start(
                        out=wo_t[:, 0:nn * 512],
                        in_=wo[h * D:(h + 1) * D, n0 * 512:n1 * 512],
                    )
                    for i in range(nn):
                        nc.tensor.matmul(
                            ps_os[i][:8, :],
                            attnT[:, h * B:(h + 1) * B],
                            wo_t[:, i * 512:(i + 1) * 512],
                            start=(h == 0), stop=(h == HP - 1),
                        )
                for i in range(nn):
                    ostage = stg.tile([8, 512], BF, tag="ostage")
                    nc.vector.tensor_copy(ostage[:], ps_os[i][:8, :])
                    if n0 + i < 12:
                        nc.sync.dma_start(
                            out=o_bounce0[:, (n0 + i) * 512:(n0 + i + 1) * 512],
                            in_=ostage[:],
                        )
                    else:
                        nc.sync.dma_start(
                            out=o_bounce1[:, (n0 + i - 12) * 512:(n0 + i - 11) * 512],
                            in_=ostage[:],
                        )
                if n1 == 12:
                    o_rs0 = dram.tile([1, 12 * 512], BF, tag="ors0")
                    nc.gpsimd.collective_compute(
                        "ReduceScatter",
                        mybir.AluOpType.add,
                        replica_groups=rg,
                        ins=[o_bounce0.opt()],
                        outs=[o_rs0.opt()],
                    )
                    nc.sync.dma_start(out=o[:, 0:12 * 512], in_=o_rs0[:])

            o_rs1 = dram.tile([1, HID - 12 * 512], BF, tag="ors1")
            nc.gpsimd.collective_compute(
                "ReduceScatter",
                mybir.AluOpType.add,
                replica_groups=rg,
                ins=[o_bounce1.opt()],
                outs=[o_rs1.opt()],
            )
            nc.sync.dma_start(out=o[:, 12 * 512:], in_=o_rs1[:])

    nc.compile()
    return nc


_NC_CACHE = None


def _get_nc():
    global _NC_CACHE
    if _NC_CACHE is None:
        _NC_CACHE = build_nc()
    return _NC_CACHE


def make_in_maps(x, k_cache, v_cache, Wq_down, Wq_up, Wkv_down, Wv_up, Wo):
    x = np.asarray(x, np.float32).reshape(B, HID)
    k_cache = np.asarray(k_cache, np.float32)
    v_cache = np.asarray(v_cache, np.float32)
    Wq_down = np.asarray(Wq_down, np.float32)
    Wq_up = np.asarray(Wq_up, np.float32)
    Wkv_down = np.asarray(Wkv_down, np.float32)
    Wv_up = np.asarray(Wv_up, np.float32)
    Wo = np.asarray(Wo, np.float32)

    np_kv = NP_FP8 if USE_FP8_KV else NP_BF16
    np_wq = NP_FP8 if USE_FP8_WQ else NP_BF16

    # xt[p, i*8+b] = x[b, i*128+p]
    xt = np.ascontiguousarray(
        x.T.reshape(56, 128, B).transpose(1, 0, 2).reshape(128, 56 * B)
    ).astype(NP_BF16)
    Wcat = np.concatenate([Wq_down, Wkv_down], axis=1)  # [7168, 2048]

    in_maps = []
    for c in range(NC_):
        hs = slice(c * HP, (c + 1) * HP)
        # w_down[p, i*COLP + n] = Wcat[i*128+p, c*COLP+n]
        wd = np.ascontiguousarray(
            Wcat[:, c * COLP:(c + 1) * COLP]
            .reshape(56, 128, COLP).transpose(1, 0, 2).reshape(128, 56 * COLP)
        ).astype(NP_BF16)
        wq = np.ascontiguousarray(
            (Wq_up[:, c * NH:(c + 1) * NH] * WQ_SCALE).reshape(12, 128, NH)
        ).astype(np_wq)
        wv = np.ascontiguousarray(
            Wv_up[:, c * NH:(c + 1) * NH].reshape(4, 128, NH)
        ).astype(NP_BF16)
        wo_c = np.ascontiguousarray(Wo[c * NH:(c + 1) * NH, :]).astype(NP_BF16)
        # kt tile g: [128 d, (u, l)] for hb=4g+u
        kt_c = np.ascontiguousarray(
            k_cache[:, hs]
            .transpose(1, 0, 3, 2)          # [h, b, d, l] (16, 8, 128, 512)
            .reshape(32, 4, 128, 512)       # [g, u, d, l]
            .transpose(0, 2, 1, 3)          # [g, d, u, l]
            .reshape(32, 128, 2048)
        ).astype(np_kv)
        # v tile g: [128 kk, (u, j, d)] for hb=4g+u, l = j*128+kk
        v_c = np.ascontiguousarray(
            v_cache[:, hs]
            .transpose(1, 0, 2, 3)          # [h, b, l, d] (16, 8, 512, 128)
            .reshape(32, 4, 4, 128, 128)    # [g, u, j, kk, d]
            .transpose(0, 3, 1, 2, 4)       # [g, kk, u, j, d]
            .reshape(32, 128, 2048)
        ).astype(np_kv)
        in_maps.append(
            {
                "xt": xt,
                "w_down": wd,
                "wq_up": wq,
                "wv_up": wv,
                "kt": kt_c,
                "v": v_c,
                "wo": wo_c,
            }
        )
    return in_maps


def kernel(x, k_cache, v_cache, Wq_down, Wq_up, Wkv_down, Wk_up, Wv_up, Wo, **_):
    in_maps = make_in_maps(
        x, k_cache, v_cache, Wq_down, Wq_up, Wkv_down, Wv_up, Wo
    )
    nc = _get_nc()
    res = bass_utils.run_bass_kernel_spmd(nc, in_maps, core_ids=list(range(NC_)))
    out = np.stack([res.results[b]["o"] for b in range(B)], axis=0)  # (8, 1, 7168)
    return np.ascontiguousarray(out, dtype=np.float32)


# revision 28
# speedup vs baseline: 1.0026x; 1.0026x over previous
"""DeepSeek-style MLA decode attention (batch=8, 128 heads, cache 512) on 8 NeuronCores.

Sharding: tensor-parallel over heads (16 heads/core).
 - down-projection [Wq_down | Wkv_down] sharded by OUTPUT column (256 cols/core);
   each core computes its cdown column slice (transposed) and a tiny AllGather
   (256x8 f32) replicates the full cdownT to every core.
 - Wq_up / Wv_up / k_cache / v_cache / Wo sharded by head.
 - o_proj partials ReduceScattered over batch (core b returns batch b's row).

All big streams are cast on the host: weights bf16, wq_up + kv caches fp8(e4m3).
Every matmul keeps the large tensor as the STATIONARY operand (it has to pass
through the PE array exactly once either way) and streams the 8-wide batch as
the moving operand, so every intermediate comes out feature-major ("pre-
transposed") and chains straight into the next matmul: no transposes and no
per-row extraction anywhere.

Note: the reference's "new token" softmax is over a length-1 axis (== 1.0), so
k_new/Wk_up are dead and the new-token contribution is simply + v_new.
"""

import numpy as np
import ml_dtypes

import concourse.bass as bass
import concourse.mybir as mybir
import concourse.tile as tile
from concourse import bacc
from concourse import bass_utils

NC_ = 8                      # cores
B = 8                        # batch
H = 128                      # total heads
HP = H // NC_                # 16 heads per core
D = 128                      # head dim
L = 512                      # cache len
HID = 7168
QL = 1536
KVL = 512
COLS = QL + KVL              # 2048 down-proj output cols
COLP = COLS // NC_           # 256 cols per core
NH = HP * D                  # 2048 per-core head cols
SCALE = 1.0 / float(np.sqrt(D))

F32 = mybir.dt.float32
BF = mybir.dt.bfloat16

# fp8 (e4m3) for the attention streams; weights are scaled on the host to sit
# in e4m3's normal range and the inverse scale is folded into the softmax /
# copy scales below.
USE_FP8_KV = True            # kt, v, qT, probsT
USE_FP8_WQ = True            # wq_up, cqT
KVDT = mybir.dt.float8e4 if USE_FP8_KV else BF
WQDT = mybir.dt.float8e4 if USE_FP8_WQ else BF
WQ_SCALE = 16.0 if USE_FP8_WQ else 1.0   # host multiplies Wq_up by this
QT_SCALE = (1.0 / 32.0) if USE_FP8_KV else 1.0  # applied when casting qT
# scoresT = (WQ_SCALE * QT_SCALE) * true_score
EXP_SCALE = SCALE / (WQ_SCALE * QT_SCALE)

NP_BF16 = ml_dtypes.bfloat16
NP_FP8 = ml_dtypes.float8_e4m3


def build_nc():
    nc = bacc.Bacc(
        "TRN2",
        target_bir_lowering=False,
        debug=False,
        enable_asserts=True,
        num_devices=NC_,
    )
    xt = nc.dram_tensor("xt", [128, 56 * B], BF, kind="ExternalInput").ap()
    w_down = nc.dram_tensor("w_down", [128, 56 * COLP], BF, kind="ExternalInput").ap()
    wq_up = nc.dram_tensor("wq_up", [12, 128, NH], WQDT, kind="ExternalInput").ap()
    wv_up = nc.dram_tensor("wv_up", [4, 128, NH], BF, kind="ExternalInput").ap()
    kt = nc.dram_tensor("kt", [32, 128, 2048], KVDT, kind="ExternalInput").ap()
    v = nc.dram_tensor("v", [32, 128, 2048], KVDT, kind="ExternalInput").ap()
    wo = nc.dram_tensor("wo", [NH, HID], BF, kind="ExternalInput").ap()
    o = nc.dram_tensor("o", [1, HID], BF, kind="ExternalOutput").ap()

    rg = [list(range(NC_))]

    with tile.TileContext(nc) as tc:
        with (
            tc.tile_pool(name="const", bufs=1) as constp,
            tc.tile_pool(name="sbuf", bufs=1) as sb,
            tc.tile_pool(name="stage", bufs=2) as stg,
            tc.tile_pool(name="wdp", bufs=4) as wdp,
            tc.tile_pool(name="wqp", bufs=12) as wqp,
            tc.tile_pool(name="wvp", bufs=4) as wvp,
            tc.tile_pool(name="ktp", bufs=22) as ktp,
            tc.tile_pool(name="vp", bufs=22) as vp,
            tc.tile_pool(name="wop", bufs=14) as wop,
            tc.tile_pool(name="psA", bufs=2, space="PSUM") as psA,
            tc.tile_pool(name="psB", bufs=6, space="PSUM") as psB,
            tc.tile_pool(name="dram", bufs=1, space="DRAM") as dram,
        ):
            # ---------------- load x and the down-proj column shard ----------------
            xt_sb = constp.tile([128, 56 * B], BF, tag="xt")
            nc.sync.dma_start(out=xt_sb[:], in_=xt[:])

            # ---------------- cdownT column slice: [COLP, B] ----------------
            # cdT[n, b] = sum_hid Wcat[hid, c*COLP+n] * x[b, hid]
            # w_down streams through a small pool: tile s holds i-chunks 4s..4s+3.
            ps_cds = [
                psB.tile([128, 512], F32, tag="bank", name=f"ps_cd{half}")
                for half in range(2)
            ]
            cd_sb = sb.tile([128, 16], F32, tag="cd")
            for s in range(14):
                wd_t = wdp.tile([128, 4 * COLP], BF, tag="wd")
                nc.sync.dma_start(
                    out=wd_t[:], in_=w_down[:, s * 4 * COLP:(s + 1) * 4 * COLP]
                )
                for q in range(4):
                    i = s * 4 + q
                    for half in range(2):
                        nc.tensor.matmul(
                            ps_cds[half][0:128, 0:B],
                            wd_t[:, q * COLP + half * 128:q * COLP + (half + 1) * 128],
                            xt_sb[:, i * B:(i + 1) * B],
                            start=(i == 0), stop=(i == 55),
                        )
            for half in range(2):
                nc.vector.tensor_copy(
                    cd_sb[:, half * B:(half + 1) * B], ps_cds[half][0:128, 0:B]
                )

            cd_bounce = dram.tile([COLP, B], F32, tag="cdb")
            nc.scalar.dma_start(
                out=cd_bounce[:].rearrange("(h p) b -> p h b", p=128),
                in_=cd_sb[:].rearrange("p (h b) -> p h b", h=2),
            )
            cd_gathered = dram.tile([COLS, B], F32, tag="cdg")
            nc.gpsimd.collective_compute(
                "AllGather",
                mybir.AluOpType.bypass,
                replica_groups=rg,
                ins=[cd_bounce.opt()],
                outs=[cd_gathered.opt()],
            )
            # cdT_sb[p, i*8+b] = cdown col (i*128+p), batch b
            cdT_sb = sb.tile([128, 128], F32, tag="cdT")
            nc.scalar.dma_start(
                out=cdT_sb[:].rearrange("p (i b) -> p i b", i=16),
                in_=cd_gathered[:].rearrange("(i p) b -> p i b", p=128),
            )
            # chunks 0..11 = q rank (1536), 12..15 = kv rank (512)
            cqT = sb.tile([128, 96], WQDT, tag="cqT")
            nc.vector.tensor_copy(cqT[:], cdT_sb[:, 0:96])
            ckvT = sb.tile([128, 32], BF, tag="ckvT")
            nc.vector.tensor_copy(ckvT[:], cdT_sb[:, 96:128])

            # ---------------- qT = (cq @ Wq_up_c)^T per head: [128 d, 16h*8b] ----------------
            ps_qT = psB.tile([128, 512], F32, tag="bank", name="ps_qT")
            wq_tiles = []
            for j in range(12):
                wq_t = wqp.tile([128, NH], WQDT, tag="wq")
                nc.sync.dma_start(out=wq_t[:], in_=wq_up[j])
                wq_tiles.append(wq_t)
            for h in range(HP):
                for j in range(12):
                    nc.tensor.matmul(
                        ps_qT[0:128, h * B:(h + 1) * B],
                        wq_tiles[j][:, h * D:(h + 1) * D],
                        cqT[:, j * B:(j + 1) * B],
                        start=(j == 0), stop=(j == 11),
                    )
            qT = sb.tile([128, 128], KVDT, tag="qT")
            nc.scalar.activation(
                qT[:], ps_qT[0:128, 0:128],
                mybir.ActivationFunctionType.Copy, scale=QT_SCALE,
            )

            # ---------------- v_newT per head: [128 d, 16h*8b] ----------------
            ps_vn = psA.tile([128, 512], F32, tag="bank", name="ps_vn")
            wv_tiles = []
            for j in range(4):
                wv_t = wvp.tile([128, NH], BF, tag="wv")
                nc.sync.dma_start(out=wv_t[:], in_=wv_up[j])
                wv_tiles.append(wv_t)
            for h in range(HP):
                for j in range(4):
                    nc.tensor.matmul(
                        ps_vn[0:128, h * B:(h + 1) * B],
                        wv_tiles[j][:, h * D:(h + 1) * D],
                        ckvT[:, j * B:(j + 1) * B],
                        start=(j == 0), stop=(j == 3),
                    )

            # ---------------- phase A: scoresT [128 k, 4j * 128 hb] ----------------
            # kt tile g holds hb=4g..4g+3 as [128 d, (u, l)]; stationary slice
            # [128 d, 128 k] per (hb, j), moving = qT column hb.
            ps_sc = psB.tile([128, 512], F32, tag="bank", name="ps_sc")
            for g in range(32):
                kt_t = ktp.tile([128, 2048], KVDT, tag="kt")
                nc.sync.dma_start(out=kt_t[:], in_=kt[g])
                for u in range(4):
                    hb = 4 * g + u
                    for j in range(4):
                        nc.tensor.matmul(
                            ps_sc[0:128, j * 128 + hb:j * 128 + hb + 1],
                            kt_t[:, u * 512 + j * 128:u * 512 + (j + 1) * 128],
                            qT[:, hb:hb + 1],
                            start=True, stop=True,
                        )

            # softmax pieces: probsT = exp(EXP_SCALE * scoresT) (unnormalized),
            # denom via ones-matmul, reciprocal broadcast to [128, hb].
            # bias shifts exp into fp8 range (max |score*scale| ~ 6); softmax
            # is shift-invariant so the denominator cancels it exactly.
            probsT = sb.tile([128, 512], KVDT, tag="probsT")
            exp_bias = constp.tile([128, 1], F32, tag="exp_bias")
            nc.vector.memset(exp_bias[:], -2.5)
            nc.scalar.activation(
                probsT[:], ps_sc[0:128, 0:512],
                mybir.ActivationFunctionType.Exp, scale=EXP_SCALE, bias=exp_bias[:],
            )
            ones_kv = constp.tile([128, 1], KVDT, tag="ones_kv")
            nc.vector.memset(ones_kv[:], 1.0)
            ps_dn = psB.tile([1, 128], F32, tag="bank", name="ps_dn")
            for j in range(4):
                nc.tensor.matmul(
                    ps_dn[0:1, 0:128],
                    ones_kv[:],
                    probsT[:, j * 128:(j + 1) * 128],
                    start=(j == 0), stop=(j == 3),
                )
            recip = sb.tile([1, 128], F32, tag="recip")
            nc.vector.reciprocal(recip[:], ps_dn[0:1, 0:128])
            ones_f32 = constp.tile([1, 128], F32, tag="ones_f32")
            nc.vector.memset(ones_f32[:], 1.0)
            ps_rb = psB.tile([128, 128], F32, tag="bank", name="ps_rb")
            nc.tensor.matmul(
                ps_rb[0:128, 0:128], ones_f32[:], recip[:], start=True, stop=True,
            )
            rb_sb = sb.tile([128, 128], F32, tag="rb_sb")
            nc.vector.tensor_copy(rb_sb[:], ps_rb[0:128, 0:128])

            # ---------------- phase B: cacheT [128 d, 128 hb] ----------------
            # v tile g holds hb=4g..4g+3 as [128 kk, (u, j, d)]; stationary
            # slice [128 kk, 128 d] per (hb, j), moving = probsT column.
            # ---------------- phase B + C round 0, interleaved per head ----------------
            # After head h's two v tiles finish, combine its attnT columns and
            # immediately emit its round-0 o_proj matmuls: the tensor queue is
            # in-order, so overlap requires interleaved EMISSION, not just
            # ready data. Remaining C rounds follow after B.
            ps_ca = psA.tile([128, 128], F32, tag="bank", name="ps_ca")
            tmp_at = sb.tile([128, 128], F32, tag="tmp_at")
            attnT = sb.tile([128, 128], BF, tag="attnT")
            o_bounce0 = dram.tile([B, 12 * 512], BF, tag="ob0")
            o_bounce1 = dram.tile([B, HID - 12 * 512], BF, tag="ob1")
            ps_os0 = [
                psB.tile([8, 512], F32, tag="bank", name=f"ps_o0_{i}")
                for i in range(4)
            ]
            for g in range(32):
                v_t = vp.tile([128, 2048], KVDT, tag="v")
                nc.sync.dma_start(out=v_t[:], in_=v[g])
                for u in range(4):
                    hb = 4 * g + u
                    for j in range(4):
                        nc.tensor.matmul(
                            ps_ca[0:128, hb:hb + 1],
                            v_t[:, (u * 4 + j) * 128:(u * 4 + j + 1) * 128],
                            probsT[:, j * 128 + hb:j * 128 + hb + 1],
                            start=(j == 0), stop=(j == 3),
                        )
                if g % 2 == 1:
                    # head h = g//2 complete: attnT_h = cacheT_h/denom + v_newT_h
                    hc = g // 2
                    cs = slice(hc * B, (hc + 1) * B)
                    nc.vector.tensor_mul(
                        tmp_at[:, cs], ps_ca[0:128, cs], rb_sb[:, cs]
                    )
                    nc.vector.tensor_add(
                        attnT[:, cs], tmp_at[:, cs], ps_vn[0:128, cs]
                    )
                    # C round 0 (cols 0:2048) for this head
                    wo_t = wop.tile([128, 2048], BF, tag="wo")
                    nc.sync.dma_start(
                        out=wo_t[:], in_=wo[hc * D:(hc + 1) * D, 0:4 * 512]
                    )
                    for i in range(4):
                        nc.tensor.matmul(
                            ps_os0[i][:8, :],
                            attnT[:, cs],
                            wo_t[:, i * 512:(i + 1) * 512],
                            start=(hc == 0), stop=(hc == HP - 1),
                        )
            for i in range(4):
                ostage = stg.tile([8, 512], BF, tag="ostage")
                nc.vector.tensor_copy(ostage[:], ps_os0[i][:8, :])
                nc.scalar.dma_start(
                    out=o_bounce0[:, i * 512:(i + 1) * 512], in_=ostage[:]
                )

            # ---------------- phase C rounds 1-3 ----------------
            for n0, n1 in ((4, 8), (8, 12), (12, 14)):
                nn = n1 - n0
                ps_os = [
                    psB.tile([8, 512], F32, tag="bank", name=f"ps_o{n0}_{i}")
                    for i in range(nn)
                ]
                for h in range(HP):
                    wo_t = wop.tile([128, 2048], BF, tag="wo")
                    nc.sync.dma_start(
                        out=wo_t[:, 0:nn * 512],
                        in_=wo[h * D:(h + 1) * D, n0 * 512:n1 * 512],
                    )
                    for i in range(nn):
                        nc.tensor.matmul(
                            ps_os[i][:8, :],
                            attnT[:, h * B:(h + 1) * B],
                            wo_t[:, i * 512:(i + 1) * 512],
                            start=(h == 0), stop=(h == HP - 1),
                        )
                for i in range(nn):
                    ostage = stg.tile([8, 512], BF, tag="ostage")
                    nc.vector.tensor_copy(ostage[:], ps_os[i][:8, :])
                    if n0 + i < 12:
                        nc.scalar.dma_start(
                            out=o_bounce0[:, (n0 + i) * 512:(n0 + i + 1) * 512],
                            in_=ostage[:],
                        )
                    else:
                        nc.scalar.dma_start(
                            out=o_bounce1[:, (n0 + i - 12) * 512:(n0 + i - 11) * 512],
                            in_=ostage[:],
                        )
                if n1 == 12:
                    o_rs0 = dram.tile([1, 12 * 512], BF, tag="ors0")
                    nc.gpsimd.collective_compute(
                        "ReduceScatter",
                        mybir.AluOpType.add,
                        replica_groups=rg,
                        ins=[o_bounce0.opt()],
                        outs=[o_rs0.opt()],
                    )
                    nc.gpsimd.dma_start(out=o[:, 0:12 * 512], in_=o_rs0[:])

            o_rs1 = dram.tile([1, HID - 12 * 512], BF, tag="ors1")
            nc.gpsimd.collective_compute(
                "ReduceScatter",
                mybir.AluOpType.add,
                replica_groups=rg,
                ins=[o_bounce1.opt()],
                outs=[o_rs1.opt()],
            )
            nc.gpsimd.dma_start(out=o[:, 12 * 512:], in_=o_rs1[:])

    nc.compile()
    return nc


_NC_CACHE = None


def _get_nc():
    global _NC_CACHE
    if _NC_CACHE is None:
        _NC_CACHE = build_nc()
    return _NC_CACHE


def make_in_maps(x, k_cache, v_cache, Wq_down, Wq_up, Wkv_down, Wv_up, Wo):
    x = np.asarray(x, np.float32).reshape(B, HID)
    k_cache = np.asarray(k_cache, np.float32)
    v_cache = np.asarray(v_cache, np.float32)
    Wq_down = np.asarray(Wq_down, np.float32)
    Wq_up = np.asarray(Wq_up, np.float32)
    Wkv_down = np.asarray(Wkv_down, np.float32)
    Wv_up = np.asarray(Wv_up, np.float32)
    Wo = np.asarray(Wo, np.float32)

    np_kv = NP_FP8 if USE_FP8_KV else NP_BF16
    np_wq = NP_FP8 if USE_FP8_WQ else NP_BF16

    # xt[p, i*8+b] = x[b, i*128+p]
    xt = np.ascontiguousarray(
        x.T.reshape(56, 128, B).transpose(1, 0, 2).reshape(128, 56 * B)
    ).astype(NP_BF16)
    Wcat = np.concatenate([Wq_down, Wkv_down], axis=1)  # [7168, 2048]

    in_maps = []
    for c in range(NC_):
        hs = slice(c * HP, (c + 1) * HP)
        # w_down[p, i*COLP + n] = Wcat[i*128+p, c*COLP+n]
        wd = np.ascontiguousarray(
            Wcat[:, c * COLP:(c + 1) * COLP]
            .reshape(56, 128, COLP).transpose(1, 0, 2).reshape(128, 56 * COLP)
        ).astype(NP_BF16)
        wq = np.ascontiguousarray(
            (Wq_up[:, c * NH:(c + 1) * NH] * WQ_SCALE).reshape(12, 128, NH)
        ).astype(np_wq)
        wv = np.ascontiguousarray(
            Wv_up[:, c * NH:(c + 1) * NH].reshape(4, 128, NH)
        ).astype(NP_BF16)
        wo_c = np.ascontiguousarray(Wo[c * NH:(c + 1) * NH, :]).astype(NP_BF16)
        # kt tile g: [128 d, (u, l)] for hb=4g+u
        kt_c = np.ascontiguousarray(
            k_cache[:, hs]
            .transpose(1, 0, 3, 2)          # [h, b, d, l] (16, 8, 128, 512)
            .reshape(32, 4, 128, 512)       # [g, u, d, l]
            .transpose(0, 2, 1, 3)          # [g, d, u, l]
            .reshape(32, 128, 2048)
        ).astype(np_kv)
        # v tile g: [128 kk, (u, j, d)] for hb=4g+u, l = j*128+kk
        v_c = np.ascontiguousarray(
            v_cache[:, hs]
            .transpose(1, 0, 2, 3)          # [h, b, l, d] (16, 8, 512, 128)
            .reshape(32, 4, 4, 128, 128)    # [g, u, j, kk, d]
            .transpose(0, 3, 1, 2, 4)       # [g, kk, u, j, d]
            .reshape(32, 128, 2048)
        ).astype(np_kv)
        in_maps.append(
            {
                "xt": xt,
                "w_down": wd,
                "wq_up": wq,
                "wv_up": wv,
                "kt": kt_c,
                "v": v_c,
                "wo": wo_c,
            }
        )
    return in_maps


def kernel(x, k_cache, v_cache, Wq_down, Wq_up, Wkv_down, Wk_up, Wv_up, Wo, **_):
    in_maps = make_in_maps(
        x, k_cache, v_cache, Wq_down, Wq_up, Wkv_down, Wv_up, Wo
    )
    nc = _get_nc()
    res = bass_utils.run_bass_kernel_spmd(nc, in_maps, core_ids=list(range(NC_)))
    out = np.stack([res.results[b]["o"] for b in range(B)], axis=0)  # (8, 1, 7168)
    return np.ascontiguousarray(out, dtype=np.float32)
